# revision 17
# baseline (speedup 1.0000x reference)
"""Additive (Bahdanau) attention on Trainium2, data-parallel over batch on 8 NeuronCores.

Per core (one batch element):
  qT[h,q] = (queries @ W_q).T, kT[h,k] = (keys @ W_k).T        (PE, bf16 in / fp32 acc)
  pre[h,(k,q)] = qT[h,q] + kT[h,k]                             (DVE tensor_tensor, bf16 2x via
                                                                pair-duplicated kt2 broadcast APs)
  feat = tanh(pre), in place                                   (ACT, big instructions - bottleneck)
  scores[q,k] = sum_h w_v[h]*feat[h,k,q]                       (PE: feat stationary, w_v moving,
                                                                one PSUM column per (k, qtile, htile))
  scores += maskbias (-1e6 where k >= valid_len)               (DVE)
  e = exp(scores), sums = row-sums via ACT accum_out           (ACT)
  out[q,:] = (e.T @ values) / sums                             (PE transpose + PE matmul + DVE)

All inputs arrive packed in two host-prepared buffers (one bf16, one f32) so the
head of the kernel is two large DMAs instead of fourteen small ones.
"""

import numpy as np

import concourse.bass as bass
import concourse.mybir as mybir
import concourse.tile as tile
from concourse import bacc
from concourse.bass_utils import run_bass_kernel_spmd

B, Q, K, H, D, DV = 8, 256, 256, 256, 256, 256
N_CORES = 8
F32 = mybir.dt.float32
BF16 = mybir.dt.bfloat16
AF = mybir.ActivationFunctionType
# chunk sizes (keys per chunk); smaller first/last chunks shorten the
# pipeline fill before the first tanh and the drain after the last one
KCS = [32, 64, 64, 64, 32]

# packed bf16 input layout (columns): kTT0 kTT1 wk0 wk1 qTT0 qTT1 wq0 wq1
NBF = 2048
# packed f32 input layout: vals0 vals1 mask identity wv0 wv1
NF32 = 256 + 256 + 256 + 128 + 2


def build_nc():
    nc = bacc.Bacc("TRN2", target_bir_lowering=False)
    d_bf = nc.dram_tensor("in_bf", [128, NBF], BF16, kind="ExternalInput")
    d_f = nc.dram_tensor("in_f32", [128, NF32], F32, kind="ExternalInput")
    d_out = nc.dram_tensor("out", [Q, DV], F32, kind="ExternalOutput")

    with tile.TileContext(nc) as tc:
        with (
            tc.tile_pool(name="sb", bufs=1) as sb,
            tc.tile_pool(name="feat", bufs=4) as feat_pool,
            tc.tile_pool(name="ps_scores", bufs=1, space=bass.MemorySpace.PSUM) as ps_s,
        ):
            # ------- packed inputs -------
            inbf = sb.tile([128, NBF], BF16, tag="inbf")
            nc.sync.dma_start(inbf[:], d_bf[:])
            inf = sb.tile([128, NF32], F32, tag="inf")
            nc.sync.dma_start(inf[:], d_f[:])
            kTT = [inbf[:, j * 256:(j + 1) * 256] for j in range(2)]
            wk_sb = [inbf[:, 512 + j * 256:512 + (j + 1) * 256] for j in range(2)]
            qTT = [inbf[:, 1024 + j * 256:1024 + (j + 1) * 256] for j in range(2)]
            wq_sb = [inbf[:, 1536 + j * 256:1536 + (j + 1) * 256] for j in range(2)]
            vals = [inf[:, t * 256:(t + 1) * 256] for t in range(2)]
            mask_sb = inf[:, 512:768]
            ident = inf[:, 768:896]
            wv_f = [inf[:, 896 + t:897 + t] for t in range(2)]

            wv_b = [sb.tile([128, 1], BF16, tag=f"wvb{t}", name=f"wvb{t}") for t in range(2)]
            qT = [sb.tile([128, Q], BF16, tag=f"qT{t}", name=f"qT{t}") for t in range(2)]
            kT = [sb.tile([128, K], BF16, tag=f"kT{t}", name=f"kT{t}") for t in range(2)]
            # kt2: each kT value duplicated into a bf16 pair -> enables TT 2x_1P packing
            kt2 = [sb.tile([128, 2 * K], BF16, tag=f"kt2{t}", name=f"kt2{t}") for t in range(2)]
            # one PSUM tile per (qt, t) so every matmul is an independent start/stop=True
            # (accumulation groups would block PE LDWEIGHTS pull-ahead and serialize MMs)
            s_ps = [[ps_s.tile([128, K], F32, tag=f"s{qt}_{t}", name=f"s{qt}_{t}")
                     for t in range(2)] for qt in range(2)]

            # ------- prep: projections (contract d); t=0 operands first -------
            with tc.tile_pool(name="ps_prep", bufs=2, space=bass.MemorySpace.PSUM) as ps_p:
                for t in range(2):
                    nc.vector.tensor_copy(wv_b[t][:], wv_f[t])
                    pk = ps_p.tile([128, 256], F32, tag="proj", name=f"pk{t}")
                    for j in range(2):
                        nc.tensor.matmul(pk[:], wk_sb[j][:, t * 128:(t + 1) * 128], kTT[j],
                                         start=(j == 0), stop=(j == 1))
                    nc.vector.tensor_copy(kT[t][:], pk[:])
                    nc.vector.tensor_copy(
                        kt2[t][:].rearrange("p (k e) -> p k e", e=2),
                        kT[t][:].unsqueeze(2).broadcast_to((128, K, 2)))
                    pq = ps_p.tile([128, 256], F32, tag="proj", name=f"pq{t}")
                    for j in range(2):
                        nc.tensor.matmul(pq[:], wq_sb[j][:, t * 128:(t + 1) * 128], qTT[j],
                                         start=(j == 0), stop=(j == 1))
                    nc.vector.tensor_copy(qT[t][:], pq[:])

            # ------- main loop: add + tanh features + w_v reduction -------
            k0 = 0
            for kc in KCS:
                for t in range(2):
                    feat = feat_pool.tile([128, kc * Q], BF16, tag="feat",
                                          padded_shape=[128, max(KCS) * Q])
                    # pre[h, j, qp, e] = qT[h, 2qp+e] + kT[h, k0+j]; bf16 pair APs keep 2x mode
                    in0 = qT[t][:].rearrange("p (qp e) -> p qp e", e=2)
                    in0 = in0.unsqueeze(1).broadcast_to((128, kc, Q // 2, 2))
                    in1 = kt2[t][:, 2 * k0:2 * (k0 + kc)].rearrange("p (k e) -> p k e", e=2)
                    in1 = in1.unsqueeze(2).broadcast_to((128, kc, Q // 2, 2))
                    out = feat[:].rearrange("p (a b c) -> p a b c", a=kc, b=Q // 2)
                    nc.vector.tensor_add(out, in0, in1)
                    nc.scalar.activation(feat[:], feat[:], AF.Tanh)
                    # t=0 matmuls overlap the t=1 add+tanh
                    for j in range(kc):
                        k = k0 + j
                        for qt in range(2):
                            nc.tensor.matmul(
                                s_ps[qt][t][:, k:k + 1],
                                feat[:, j * Q + qt * 128: j * Q + qt * 128 + 128],
                                wv_b[t][:],
                                start=True, stop=True)
                k0 += kc

            # ------- masked softmax + attention @ values -------
            exp_sb = [sb.tile([128, K], F32, tag=f"exp{qt}", name=f"exp{qt}") for qt in range(2)]
            expT = [sb.tile([128, Q], F32, tag=f"expT{kt}", name=f"expT{kt}") for kt in range(2)]
            sums = [sb.tile([128, 1], F32, tag=f"sum{qt}", name=f"sum{qt}") for qt in range(2)]
            recip = [sb.tile([128, 1], F32, tag=f"rcp{qt}", name=f"rcp{qt}") for qt in range(2)]
            out_sb = [sb.tile([128, DV], F32, tag=f"out{qt}", name=f"out{qt}") for qt in range(2)]
            with tc.tile_pool(name="ps_tail", bufs=2, space=bass.MemorySpace.PSUM) as ps_t:
                for qt in range(2):
                    # TT may read only one PSUM operand: stage s1+mask into SBUF first
                    nc.vector.tensor_add(exp_sb[qt][:], s_ps[qt][1][:], mask_sb)
                    nc.vector.tensor_add(s_ps[qt][0][:], s_ps[qt][0][:], exp_sb[qt][:])
                    nc.scalar.activation(exp_sb[qt][:], s_ps[qt][0][:], AF.Exp,
                                         accum_out=sums[qt][:])
                for qt in range(2):
                    for kt in range(2):
                        tx = ps_t.tile([128, 128], F32, tag="tx")
                        nc.tensor.transpose(tx[:], exp_sb[qt][:, kt * 128:(kt + 1) * 128], ident)
                        nc.vector.tensor_copy(expT[kt][:, qt * 128:(qt + 1) * 128], tx[:])
                for qt in range(2):
                    av = ps_t.tile([128, DV], F32, tag="av")
                    for kt in range(2):
                        nc.tensor.matmul(av[:], expT[kt][:, qt * 128:(qt + 1) * 128], vals[kt],
                                         start=(kt == 0), stop=(kt == 1))
                    nc.vector.reciprocal(recip[qt][:], sums[qt][:])
                    nc.vector.tensor_scalar_mul(out_sb[qt][:], av[:], recip[qt][:])
                    nc.sync.dma_start(d_out[qt * 128:(qt + 1) * 128, :], out_sb[qt][:])
    nc.compile()
    return nc


_NC = None


def _get_nc():
    global _NC
    if _NC is None:
        _NC = build_nc()
    return _NC


def _make_in_maps(queries, keys, values, valid_lens, W_q, W_k, w_v):
    import ml_dtypes
    bf16 = ml_dtypes.bfloat16
    queries = np.asarray(queries, dtype=np.float32)
    keys = np.asarray(keys, dtype=np.float32)
    values = np.asarray(values, dtype=np.float32)
    valid_lens = np.asarray(valid_lens)
    W_q = np.asarray(W_q, dtype=np.float32)
    W_k = np.asarray(W_k, dtype=np.float32)
    w_v = np.asarray(w_v, dtype=np.float32).reshape(H)
    ident = np.eye(128, dtype=np.float32)
    arange = np.arange(K)

    wkb = W_k.astype(bf16)
    wqb = W_q.astype(bf16)
    in_maps = []
    for b in range(B):
        kTTb = keys[b].T.astype(bf16)     # [D, K]
        qTTb = queries[b].T.astype(bf16)  # [D, Q]
        in_bf = np.empty((128, NBF), dtype=bf16)
        in_bf[:, 0:256] = kTTb[0:128]
        in_bf[:, 256:512] = kTTb[128:256]
        in_bf[:, 512:768] = wkb[0:128]
        in_bf[:, 768:1024] = wkb[128:256]
        in_bf[:, 1024:1280] = qTTb[0:128]
        in_bf[:, 1280:1536] = qTTb[128:256]
        in_bf[:, 1536:1792] = wqb[0:128]
        in_bf[:, 1792:2048] = wqb[128:256]

        vl = int(valid_lens[b])
        maskrow = np.where(arange >= vl, np.float32(-1.0e6), np.float32(0.0))
        in_f = np.empty((128, NF32), dtype=np.float32)
        in_f[:, 0:256] = values[b][0:128]
        in_f[:, 256:512] = values[b][128:256]
        in_f[:, 512:768] = maskrow[None, :]
        in_f[:, 768:896] = ident
        in_f[:, 896] = w_v[0:128]
        in_f[:, 897] = w_v[128:256]
        in_maps.append({"in_bf": in_bf, "in_f32": in_f})
    return in_maps


def run_spmd(in_maps, **kwargs):
    nc = _get_nc()
    return run_bass_kernel_spmd(nc, in_maps, core_ids=list(range(N_CORES)), **kwargs)


def kernel(queries, keys, values, valid_lens, W_q, W_k, w_v):
    in_maps = _make_in_maps(queries, keys, values, valid_lens, W_q, W_k, w_v)
    res = run_spmd(in_maps)
    return np.stack([res.results[b]["out"] for b in range(B)]).astype(np.float32)
